# revision 15
# baseline (speedup 1.0000x reference)
"""MoE-routed autoencoder (4 experts, 1024->512->128->512->1024) on 8 TRN2 cores.

v2 strategy (f16 matmuls at the PE roofline):
- Host: sort atoms by expert, deal evenly across 8 cores. Per expert:
  ceil(cap/512) equal-width tiles (multiples of 8 cols); the last tile
  OVERLAPS the previous one to absorb rounding (overlap atoms computed
  twice). Equal widths avoid narrow tiles, whose matmuls expose the
  ~128-cycle PE pipeline drain. Tiles run widest-first so the final y DMA
  is the smallest.
- Device: transposed activations [feat, atoms]; all four experts' weights are
  preloaded into SBUF up front (bulk rides the scalar/gpsimd DMA queues so
  the sync queue stays free for x prefetches). Tile 0's L1 runs k-outer so
  compute starts after a single k-chunk of w1/x has landed.
- Pipeline: the eviction->consume latency at each layer boundary (~1us) is
  hidden by interleaving the NEXT tile's L1 m-chunk chains between this
  tile's L2 / L3 / L4 groups, so the PE instruction stream never waits on a
  fresh eviction.
- Evictions (relu/copy, fused f32->f16) round-robin over scalar+vector (the
  two PSUM-capable engines). PSUM banks: 4 = L1 chains, 1 = L2 (+L3 m3),
  3 = L3 m0-2 + L4 rotation.
- All SBUF tiles are 2-D [128, free] and DMAs move 2-D contiguous views
  (8KB per partition) -- 3-D access patterns get chopped into 1KB DMA
  packets and halve the effective HBM bandwidth.
- Output evicted as f16 (halves the out-DMA; adds ~3e-4 rel err), host casts
  back to f32 and scatters to original atom order.
"""

import numpy as np

N_CORES = 8

_PROGRAM_CACHE: dict = {}

# test-harness knobs: when _TRACE is set, the SPMD launch requests an NTFF
# profile and the BassKernelResults lands in _LAST["res"].
_TRACE = False
_LAST: dict = {}


def _plan(dims, tiles):
    """Flat-buffer offsets for the tile-order X / Y layouts.

    tiles[e] = [T_0, T_1, ...] per-tile widths.
    Returns seq of (e, t, T, xoff, yoff): full tiles first, tiny tiles last."""
    D_IN, H1, LAT, D_OUT, E = dims
    KC1 = D_IN // 128
    MC4 = D_OUT // 128
    entries = []
    for e in range(E):
        for t, T in enumerate(tiles[e]):
            entries.append((e, t, T))
    entries.sort(key=lambda et: -et[2])
    xoff, yoff, seq = 0, 0, []
    for e, t, T in entries:
        seq.append((e, t, T, xoff, yoff))
        xoff += 128 * KC1 * T
        yoff += 128 * MC4 * T
    return seq, xoff, yoff


def _build_program(dims, tiles, use_bias, n_bias_cols):
    import concourse.bass as bass  # noqa: F401
    import concourse.tile as tile
    from concourse import bacc, mybir

    D_IN, H1, LAT, D_OUT, E = dims
    f32 = mybir.dt.float32
    f16 = mybir.dt.float16
    RELU = mybir.ActivationFunctionType.Relu
    COPY = mybir.ActivationFunctionType.Copy
    ADD = mybir.AluOpType.add
    MAX = mybir.AluOpType.max

    KC1 = D_IN // 128   # 8
    MC1 = H1 // 128     # 4
    KC2 = H1 // 128     # 4
    KC4 = H1 // 128     # 4
    MC4 = D_OUT // 128  # 8
    NBC = MC1 + 1 + MC1 + MC4  # bias cols per expert (17)

    seq, x_total, y_total = _plan(dims, tiles)
    nseq = len(seq)

    # all four layers' weights packed per expert: one DMA per expert
    W1O = 0
    W2O = W1O + KC1 * H1
    W3O = W2O + KC2 * LAT
    W4O = W3O + H1
    WCOLS = W4O + KC4 * D_OUT

    nc = bacc.Bacc("TRN2", target_bir_lowering=False, debug=False,
                   num_devices=N_CORES)
    xt = nc.dram_tensor("xt", [x_total], f16, kind="ExternalInput").ap()
    wall = nc.dram_tensor("wall", [E, 128, WCOLS], f16,
                          kind="ExternalInput").ap()
    if use_bias:
        bias = nc.dram_tensor("bias", [128, n_bias_cols], f32,
                              kind="ExternalInput").ap()
    yt = nc.dram_tensor("yt", [y_total], f16, kind="ExternalOutput").ap()

    def xview(xo, T):
        return xt[xo:xo + 128 * KC1 * T].rearrange("(p f) -> p f", p=128)

    with tile.TileContext(nc) as tc:
        with (
            tc.tile_pool(name="wp", bufs=1) as wp,
            tc.tile_pool(name="xp", bufs=5) as xp,
            tc.tile_pool(name="hp", bufs=2) as hp,
            tc.tile_pool(name="zp", bufs=2) as zp,
            tc.tile_pool(name="dp", bufs=2) as dp,
            tc.tile_pool(name="yp", bufs=2) as yp,
            tc.tile_pool(name="aux", bufs=1) as aux,
            tc.tile_pool(name="ppa", bufs=4, space="PSUM") as ppa,
            tc.tile_pool(name="ppb", bufs=1, space="PSUM") as ppb,
            tc.tile_pool(name="ppc", bufs=3, space="PSUM") as ppc,
        ):
            if use_bias:
                btile = aux.tile([128, n_bias_cols], f32, name="btile")
                nc.gpsimd.dma_start(btile[:], bias[:])

            # round-robin eviction across the two PSUM-capable engines
            # (gpsimd cannot read PSUM)
            ev_i = [0]

            def evict(out_ap, ps_ap, relu, bcol):
                i = ev_i[0] % 2
                ev_i[0] += 1
                if use_bias:
                    b = btile[:, bcol:bcol + 1]
                    if i == 0:
                        nc.scalar.activation(out_ap, ps_ap,
                                             RELU if relu else COPY, bias=b)
                    elif relu:
                        nc.vector.tensor_scalar(out_ap, ps_ap, b, 0.0,
                                                ADD, MAX)
                    else:
                        nc.vector.tensor_scalar_add(out_ap, ps_ap, b)
                elif i == 0:
                    nc.scalar.activation(out_ap, ps_ap, RELU if relu else COPY)
                elif relu:
                    nc.vector.tensor_scalar_max(out_ap, ps_ap, 0.0)
                else:
                    nc.vector.tensor_copy(out_ap, ps_ap)

            # ---- weight tiles (all experts preloaded, packed) ----
            wt = {}
            for e in range(E):
                wt[e] = wp.tile([128, WCOLS], f16, name=f"we{e}")

            # ---- PE warmup: dependency-free matmuls release the clock gate.
            # Sized to keep the PE continuously busy until the first tile's
            # w1/x chunks land (~12us): an idle gap here resets the p-state
            # ramp and the first real chains run at the low clock.
            warm = aux.tile([128, 128], f16, name="warm")
            nc.vector.memset(warm[:], 0.0)
            wps = ppc.tile([128, 128], f32, tag="ps", name="warmps")
            for _ in range(40):
                nc.tensor.matmul(wps[:], warm[:], warm[:],
                                 start=True, stop=True)

            # ---- cold-start DMA. Each dma_start costs ~0.7us of issue time
            # on its engine, so keep the counts low and the queues parallel:
            #   sync ring:   x tiles only (x0 split in halves for a fast L1
            #                start, then x1..x3)
            #   scalar ring: w1/w2 of the first expert (halves), then bulk
            #   gpsimd ring: w3/w4 (+ bias), later the y stores
            e0 = seq[0][0]
            T0 = seq[0][2]
            x_tiles = {}
            xv0 = xview(0, T0)
            xt0 = xp.tile([128, KC1 * T0], f16, tag="x", name="xtile0")
            # first-expert weights + x0 land in k-chunk groups [0:2),[2:4),
            # [4:8) so the k-outer L1 starts after only 2 k-chunks
            for a, b in ((0, 2), (2, 4), (4, KC1)):
                nc.sync.dma_start(xt0[:, a * T0:b * T0], xv0[:, a * T0:b * T0])
                nc.scalar.dma_start(wt[e0][:, a * H1:b * H1],
                                    wall[e0][:, a * H1:b * H1])
            nc.scalar.dma_start(wt[e0][:, KC1 * H1:], wall[e0][:, KC1 * H1:])
            x_tiles[0] = xt0
            for si in range(1, min(4, nseq)):
                _, _, Ti, xoi, _ = seq[si]
                xn = xp.tile([128, KC1 * Ti], f16, tag="x",
                             name=f"xtile{si}")
                nc.sync.dma_start(xn[:], xview(xoi, Ti))
                x_tiles[si] = xn
            worder = []
            for e, t, T, xo, yo in seq:
                if e != e0 and e not in worder:
                    worder.append(e)
            for e in worder:
                nc.scalar.dma_start(wt[e][:], wall[e])

            h_tiles = {}

            def l1_chain(si, m):
                """One L1 m-chunk chain (8 matmuls) + its eviction."""
                e, t, T, xo, yo = seq[si]
                if si not in h_tiles:
                    h_tiles[si] = hp.tile([128, MC1 * T], f16, tag="h",
                                          name=f"h{si}")
                htile = h_tiles[si]
                ps = ppa.tile([128, T], f32, tag="ps", name=f"ps1_{si}_{m}")
                for k in range(KC1):
                    nc.tensor.matmul(
                        ps[:],
                        wt[e][:, W1O + k * H1 + m * 128:
                               W1O + k * H1 + (m + 1) * 128],
                        x_tiles[si][:, k * T:(k + 1) * T],
                        start=(k == 0), stop=(k == KC1 - 1))
                evict(htile[:, m * T:(m + 1) * T], ps[:], True, e * NBC + m)

            def l1_tile0():
                """Tile 0 L1, k-outer: starts on the first w1/x k-chunk."""
                e, t, T, xo, yo = seq[0]
                h_tiles[0] = hp.tile([128, MC1 * T], f16, tag="h", name="h0")
                pss = [ppa.tile([128, T], f32, tag="ps", name=f"ps1_0_{m}")
                       for m in range(MC1)]
                for k in range(KC1):
                    for m in range(MC1):
                        nc.tensor.matmul(
                            pss[m][:],
                            wt[e][:, W1O + k * H1 + m * 128:
                               W1O + k * H1 + (m + 1) * 128],
                            x_tiles[0][:, k * T:(k + 1) * T],
                            start=(k == 0), stop=(k == KC1 - 1))
                for m in range(MC1):
                    evict(h_tiles[0][:, m * T:(m + 1) * T], pss[m][:], True,
                          e * NBC + m)

            l1_tile0()

            # ---- main pipeline: iteration si emits L2/L3/L4(si) with
            # L1(si+1) m-chunk chains interleaved as latency filler ----
            for si, (e, t, T, xo, yo) in enumerate(seq):
                # prefetch x four tiles ahead (x0..x3 issued in cold start)
                pi = si + 4
                if pi < nseq:
                    _, _, Tp, xop, _ = seq[pi]
                    xn = xp.tile([128, KC1 * Tp], f16, tag="x",
                                 name=f"xtile{pi}")
                    nc.sync.dma_start(xn[:], xview(xop, Tp))
                    x_tiles[pi] = xn

                fill = (lambda m: l1_chain(si + 1, m)) if si + 1 < nseq \
                    else (lambda m: None)

                fill(0)

                # L2: z[LAT, T] = relu(W2.T @ h)
                htile = h_tiles.pop(si)
                ztile = zp.tile([128, T], f16, tag="z", name=f"z{si}")
                ps2 = ppb.tile([128, T], f32, tag="ps", name=f"ps2_{si}")
                for k in range(KC2):
                    nc.tensor.matmul(ps2[:],
                                     wt[e][:, W2O + k * LAT:W2O + (k + 1) * LAT],
                                     htile[:, k * T:(k + 1) * T],
                                     start=(k == 0), stop=(k == KC2 - 1))
                evict(ztile[:], ps2[:], True, e * NBC + MC1)

                fill(1)

                # L3: d[H1, T] = relu(W3.T @ z)  (K=128: single-matmul chunks)
                dtile = dp.tile([128, MC1 * T], f16, tag="d", name=f"d{si}")
                for m in range(MC1):
                    pool = ppb if m == MC1 - 1 else ppc
                    ps3 = pool.tile([128, T], f32, tag="ps",
                                    name=f"ps3_{si}_{m}")
                    nc.tensor.matmul(ps3[:],
                                     wt[e][:, W3O + m * 128:W3O + (m + 1) * 128],
                                     ztile[:], start=True, stop=True)
                    evict(dtile[:, m * T:(m + 1) * T], ps3[:], True,
                          e * NBC + MC1 + 1 + m)

                fill(2)

                # L4: y[D_OUT, T] = W4.T @ d (no relu), chains of 4
                ytile = yp.tile([128, MC4 * T], f16, tag="y", name=f"y{si}")
                yv = yt[yo:yo + 128 * MC4 * T].rearrange("(p f) -> p f",
                                                         p=128)
                for m in range(MC4):
                    ps4 = ppc.tile([128, T], f32, tag="ps",
                                   name=f"ps4_{si}_{m}")
                    for k in range(KC4):
                        nc.tensor.matmul(
                            ps4[:],
                            wt[e][:, W4O + k * D_OUT + m * 128:
                                  W4O + k * D_OUT + (m + 1) * 128],
                            dtile[:, k * T:(k + 1) * T],
                            start=(k == 0), stop=(k == KC4 - 1))
                    evict(ytile[:, m * T:(m + 1) * T], ps4[:], False,
                          e * NBC + 2 * MC1 + 1 + m)
                if si == nseq - 1:
                    # last tile: store in halves on the (idle by then) sync
                    # queue so the first half overlaps the m4-7 evictions
                    hcols = MC4 // 2 * T
                    nc.sync.dma_start(yv[:, :hcols], ytile[:, :hcols])
                    nc.sync.dma_start(yv[:, hcols:], ytile[:, hcols:])
                else:
                    nc.gpsimd.dma_start(yv[:], ytile[:])

                fill(3)

    nc.compile()
    return nc


def kernel(**inputs) -> np.ndarray:
    from concourse.bass_utils import run_bass_kernel_spmd

    X = np.ascontiguousarray(inputs["X"], dtype=np.float32)
    sym_ids = np.asarray(inputs["sym_ids"]).astype(np.int64).ravel()
    We = [inputs["We1"], inputs["We2"], inputs["Wd1"], inputs["Wd2"]]
    be = [np.asarray(inputs["be1"], dtype=np.float32),
          np.asarray(inputs["be2"], dtype=np.float32),
          np.asarray(inputs["bd1"], dtype=np.float32),
          np.asarray(inputs["bd2"], dtype=np.float32)]
    use_bias = any(np.any(b) for b in be)

    N, D_IN = X.shape
    E, _, H1 = We[0].shape
    LAT = We[1].shape[2]
    D_OUT = We[3].shape[2]
    KC1 = D_IN // 128
    MC1 = H1 // 128
    MC4 = D_OUT // 128
    NBC = MC1 + 1 + MC1 + MC4
    TMAX = 512

    # ---- host routing: per-expert, per-core index assignment ----
    core_idx = [[None] * E for _ in range(N_CORES)]
    C_e = [0] * E
    for e in range(E):
        idx = np.flatnonzero(sym_ids == e)
        n = len(idx)
        base, rem = divmod(n, N_CORES)
        s = 0
        for c in range(N_CORES):
            cnt = base + (1 if c < rem else 0)
            core_idx[c][e] = idx[s:s + cnt]
            s += cnt
        C_e[e] = base + (1 if rem else 0)

    # per-expert tiling: nt equal-width tiles (multiples of 8); the last tile
    # overlaps backwards into the previous one to absorb the rounding. Equal
    # widths avoid narrow tiles, whose matmuls expose the ~128-cycle PE
    # pipeline drain per instruction.
    tiles = []
    for e in range(E):
        ce = max(C_e[e], 1)
        nt = -(-ce // TMAX)
        Tw = -(-ce // nt)
        T = -(-Tw // 8) * 8
        tiles.append(tuple([T] * nt))

    # ---- build / fetch compiled program ----
    dims = (D_IN, H1, LAT, D_OUT, E)
    n_bias_cols = E * NBC
    key = (dims, tuple(tiles), use_bias)
    nc = _PROGRAM_CACHE.get(key)
    if nc is None:
        nc = _build_program(dims, tiles, use_bias, n_bias_cols)
        _PROGRAM_CACHE[key] = nc

    # ---- prepare inputs ----
    XrT = np.ascontiguousarray(X.astype(np.float16).T)       # [D_IN, N]

    # weights in device layout: [E, 128, kc*mw] (k-chunk-major columns)
    def wdev(w, kc, mw):
        return np.ascontiguousarray(
            np.asarray(w, dtype=np.float32).astype(np.float16)
            .reshape(E, kc, 128, mw).transpose(0, 2, 1, 3)
            .reshape(E, 128, kc * mw))

    Wall = np.ascontiguousarray(np.concatenate(
        [wdev(We[0], KC1, H1), wdev(We[1], H1 // 128, LAT),
         wdev(We[2], LAT // 128, H1), wdev(We[3], H1 // 128, D_OUT)],
        axis=2))

    bias_h = None
    if use_bias:
        bias_h = np.zeros((128, n_bias_cols), np.float32)
        for e in range(E):
            col = e * NBC
            for b in (be[0][e], be[1][e], be[2][e], be[3][e]):
                for mch in range(len(b) // 128):
                    bias_h[:, col] = b[mch * 128:(mch + 1) * 128]
                    col += 1

    seq, x_total, y_total = _plan(dims, tiles)

    perms = []
    in_maps = []
    for c in range(N_CORES):
        xflat = np.empty(x_total, dtype=np.float16)
        perm = []
        for e, t, T, xo, yo in seq:
            idx = core_idx[c][e]
            ce = len(idx)
            start = max(min(t * T, ce - T), 0)
            rows = idx[start:start + T]
            nvalid = len(rows)
            if nvalid < T:  # tiny core count: pad with first atom
                rows = np.concatenate(
                    [rows, np.full(T - nvalid, idx[0] if ce else 0,
                                   np.int64)])
            perm.append((rows, nvalid))
            # [D_IN, T] -> [128, KC1, T] (k-chunk major per partition)
            g = XrT[:, rows].reshape(KC1, 128, T)
            xflat[xo:xo + 128 * KC1 * T] = (
                g.transpose(1, 0, 2).reshape(-1))
        perms.append(perm)
        m = {"xt": xflat, "wall": Wall}
        if use_bias:
            m["bias"] = bias_h
        in_maps.append(m)

    res = run_bass_kernel_spmd(nc, in_maps, core_ids=list(range(N_CORES)),
                               trace=_TRACE)
    _LAST["res"] = res

    # ---- unshard ----
    Y = np.empty((N, D_OUT), dtype=np.float32)
    for c in range(N_CORES):
        yflat = res.results[c]["yt"]
        for ti, (e, t, T, xo, yo) in enumerate(seq):
            ytc = (yflat[yo:yo + 128 * MC4 * T].astype(np.float32)
                   .reshape(128, MC4, T).transpose(1, 0, 2)
                   .reshape(D_OUT, T))
            rows, nvalid = perms[c][ti]
            Y[rows[:nvalid]] = ytc.T[:nvalid]
    return Y


# revision 16
# speedup vs baseline: 1.2019x; 1.2019x over previous
"""MoE-routed autoencoder (4 experts, 1024->512->128->512->1024) on 8 TRN2 cores.

v2 strategy (f16 matmuls at the PE roofline):
- Host: sort atoms by expert, deal evenly across 8 cores. Per expert:
  ceil(cap/512) equal-width tiles (multiples of 8 cols); the last tile
  OVERLAPS the previous one to absorb rounding (overlap atoms computed
  twice). Equal widths avoid narrow tiles, whose matmuls expose the
  ~128-cycle PE pipeline drain. Tiles run widest-first so the final y DMA
  is the smallest.
- Device: transposed activations [feat, atoms]; all four experts' weights are
  preloaded into SBUF up front (bulk rides the scalar/gpsimd DMA queues so
  the sync queue stays free for x prefetches). Tile 0's L1 runs k-outer so
  compute starts after a single k-chunk of w1/x has landed.
- Pipeline: the eviction->consume latency at each layer boundary (~1us) is
  hidden by interleaving the NEXT tile's L1 m-chunk chains between this
  tile's L2 / L3 / L4 groups, so the PE instruction stream never waits on a
  fresh eviction.
- Evictions (relu/copy, fused f32->f16) round-robin over scalar+vector (the
  two PSUM-capable engines). PSUM banks: 4 = L1 chains, 1 = L2 (+L3 m3),
  3 = L3 m0-2 + L4 rotation.
- All SBUF tiles are 2-D [128, free] and DMAs move 2-D contiguous views
  (8KB per partition) -- 3-D access patterns get chopped into 1KB DMA
  packets and halve the effective HBM bandwidth.
- Output evicted as f16 (halves the out-DMA; adds ~3e-4 rel err), host casts
  back to f32 and scatters to original atom order.
"""

import numpy as np

N_CORES = 8

_PROGRAM_CACHE: dict = {}

# test-harness knobs: when _TRACE is set, the SPMD launch requests an NTFF
# profile and the BassKernelResults lands in _LAST["res"].
_TRACE = False
_LAST: dict = {}


def _plan(dims, tiles):
    """Flat-buffer offsets for the tile-order X / Y layouts.

    tiles[e] = [T_0, T_1, ...] per-tile widths.
    Returns seq of (e, t, T, xoff, yoff): full tiles first, tiny tiles last."""
    D_IN, H1, LAT, D_OUT, E = dims
    KC1 = D_IN // 128
    MC4 = D_OUT // 128
    entries = []
    for e in range(E):
        for t, T in enumerate(tiles[e]):
            entries.append((e, t, T))
    entries.sort(key=lambda et: -et[2])
    xoff, yoff, seq = 0, 0, []
    for e, t, T in entries:
        seq.append((e, t, T, xoff, yoff))
        xoff += 128 * KC1 * T
        yoff += 128 * MC4 * T
    return seq, xoff, yoff


def _build_program(dims, tiles, use_bias, n_bias_cols):
    import concourse.bass as bass  # noqa: F401
    import concourse.tile as tile
    from concourse import bacc, mybir

    D_IN, H1, LAT, D_OUT, E = dims
    f32 = mybir.dt.float32
    f16 = mybir.dt.float16
    RELU = mybir.ActivationFunctionType.Relu
    COPY = mybir.ActivationFunctionType.Copy
    ADD = mybir.AluOpType.add
    MAX = mybir.AluOpType.max

    KC1 = D_IN // 128   # 8
    MC1 = H1 // 128     # 4
    KC2 = H1 // 128     # 4
    KC4 = H1 // 128     # 4
    MC4 = D_OUT // 128  # 8
    NBC = MC1 + 1 + MC1 + MC4  # bias cols per expert (17)

    seq, x_total, y_total = _plan(dims, tiles)
    nseq = len(seq)

    # all four layers' weights packed per expert: one DMA per expert
    W1O = 0
    W2O = W1O + KC1 * H1
    W3O = W2O + KC2 * LAT
    W4O = W3O + H1
    WCOLS = W4O + KC4 * D_OUT

    nc = bacc.Bacc("TRN2", target_bir_lowering=False, debug=False,
                   num_devices=N_CORES)
    xt = nc.dram_tensor("xt", [x_total], f16, kind="ExternalInput").ap()
    wall = nc.dram_tensor("wall", [E, 128, WCOLS], f16,
                          kind="ExternalInput").ap()
    if use_bias:
        bias = nc.dram_tensor("bias", [128, n_bias_cols], f32,
                              kind="ExternalInput").ap()
    yt = nc.dram_tensor("yt", [y_total], f16, kind="ExternalOutput").ap()

    def xview(xo, T):
        return xt[xo:xo + 128 * KC1 * T].rearrange("(p f) -> p f", p=128)

    with tile.TileContext(nc) as tc:
        with (
            tc.tile_pool(name="wp", bufs=1) as wp,
            tc.tile_pool(name="xp", bufs=5) as xp,
            tc.tile_pool(name="hp", bufs=2) as hp,
            tc.tile_pool(name="zp", bufs=2) as zp,
            tc.tile_pool(name="dp", bufs=2) as dp,
            tc.tile_pool(name="yp", bufs=2) as yp,
            tc.tile_pool(name="aux", bufs=1) as aux,
            tc.tile_pool(name="ppa", bufs=4, space="PSUM") as ppa,
            tc.tile_pool(name="ppb", bufs=1, space="PSUM") as ppb,
            tc.tile_pool(name="ppc", bufs=3, space="PSUM") as ppc,
        ):
            if use_bias:
                btile = aux.tile([128, n_bias_cols], f32, name="btile")
                nc.gpsimd.dma_start(btile[:], bias[:])

            # round-robin eviction across the two PSUM-capable engines
            # (gpsimd cannot read PSUM)
            ev_i = [0]

            def evict(out_ap, ps_ap, relu, bcol):
                i = ev_i[0] % 2
                ev_i[0] += 1
                if use_bias:
                    b = btile[:, bcol:bcol + 1]
                    if i == 0:
                        nc.scalar.activation(out_ap, ps_ap,
                                             RELU if relu else COPY, bias=b)
                    elif relu:
                        nc.vector.tensor_scalar(out_ap, ps_ap, b, 0.0,
                                                ADD, MAX)
                    else:
                        nc.vector.tensor_scalar_add(out_ap, ps_ap, b)
                elif i == 0:
                    nc.scalar.activation(out_ap, ps_ap, RELU if relu else COPY)
                elif relu:
                    nc.vector.tensor_scalar_max(out_ap, ps_ap, 0.0)
                else:
                    nc.vector.tensor_copy(out_ap, ps_ap)

            # ---- weight tiles (all experts preloaded, packed) ----
            wt = {}
            for e in range(E):
                wt[e] = wp.tile([128, WCOLS], f16, name=f"we{e}")

            # ---- PE warmup: dependency-free matmuls release the clock gate.
            # Sized to keep the PE continuously busy until the first tile's
            # w1/x chunks land (~12us): an idle gap here resets the p-state
            # ramp and the first real chains run at the low clock.
            warm = aux.tile([128, 128], f16, name="warm")
            nc.vector.memset(warm[:], 0.0)
            wps = ppc.tile([128, 128], f32, tag="ps", name="warmps")
            for _ in range(40):
                nc.tensor.matmul(wps[:], warm[:], warm[:],
                                 start=True, stop=True)

            # ---- cold-start DMA. Each dma_start costs ~0.7us of issue time
            # on its engine, so keep the counts low and the queues parallel:
            #   sync ring:   x tiles only (x0 split in halves for a fast L1
            #                start, then x1..x3)
            #   scalar ring: w1/w2 of the first expert (halves), then bulk
            #   gpsimd ring: w3/w4 (+ bias), later the y stores
            e0 = seq[0][0]
            T0 = seq[0][2]
            x_tiles = {}
            xv0 = xview(0, T0)
            xt0 = xp.tile([128, KC1 * T0], f16, tag="x", name="xtile0")
            # first-expert weights + x0 land in k-chunk groups [0:2),[2:4),
            # [4:8) so the k-outer L1 starts after only 2 k-chunks
            for a, b in ((0, 2), (2, 4), (4, KC1)):
                nc.sync.dma_start(xt0[:, a * T0:b * T0], xv0[:, a * T0:b * T0])
                nc.scalar.dma_start(wt[e0][:, a * H1:b * H1],
                                    wall[e0][:, a * H1:b * H1])
            nc.scalar.dma_start(wt[e0][:, KC1 * H1:], wall[e0][:, KC1 * H1:])
            x_tiles[0] = xt0
            for si in range(1, min(4, nseq)):
                _, _, Ti, xoi, _ = seq[si]
                xn = xp.tile([128, KC1 * Ti], f16, tag="x",
                             name=f"xtile{si}")
                nc.sync.dma_start(xn[:], xview(xoi, Ti))
                x_tiles[si] = xn
            worder = []
            for e, t, T, xo, yo in seq:
                if e != e0 and e not in worder:
                    worder.append(e)
            for e in worder:
                nc.scalar.dma_start(wt[e][:], wall[e])

            h_tiles = {}

            def l1_chain(si, m):
                """One L1 m-chunk chain (8 matmuls) + its eviction."""
                e, t, T, xo, yo = seq[si]
                if si not in h_tiles:
                    h_tiles[si] = hp.tile([128, MC1 * T], f16, tag="h",
                                          name=f"h{si}")
                htile = h_tiles[si]
                ps = ppa.tile([128, T], f32, tag="ps", name=f"ps1_{si}_{m}")
                for k in range(KC1):
                    nc.tensor.matmul(
                        ps[:],
                        wt[e][:, W1O + k * H1 + m * 128:
                               W1O + k * H1 + (m + 1) * 128],
                        x_tiles[si][:, k * T:(k + 1) * T],
                        start=(k == 0), stop=(k == KC1 - 1))
                evict(htile[:, m * T:(m + 1) * T], ps[:], True, e * NBC + m)

            def l1_tile0():
                """Tile 0 L1, k-outer: starts on the first w1/x k-chunk."""
                e, t, T, xo, yo = seq[0]
                h_tiles[0] = hp.tile([128, MC1 * T], f16, tag="h", name="h0")
                pss = [ppa.tile([128, T], f32, tag="ps", name=f"ps1_0_{m}")
                       for m in range(MC1)]
                for k in range(KC1):
                    for m in range(MC1):
                        nc.tensor.matmul(
                            pss[m][:],
                            wt[e][:, W1O + k * H1 + m * 128:
                               W1O + k * H1 + (m + 1) * 128],
                            x_tiles[0][:, k * T:(k + 1) * T],
                            start=(k == 0), stop=(k == KC1 - 1))
                for m in range(MC1):
                    evict(h_tiles[0][:, m * T:(m + 1) * T], pss[m][:], True,
                          e * NBC + m)

            l1_tile0()

            # ---- main pipeline: iteration si emits L2/L3/L4(si) with
            # L1(si+1) m-chunk chains interleaved as latency filler ----
            for si, (e, t, T, xo, yo) in enumerate(seq):
                # prefetch x four tiles ahead (x0..x3 issued in cold start)
                pi = si + 4
                if pi < nseq:
                    _, _, Tp, xop, _ = seq[pi]
                    xn = xp.tile([128, KC1 * Tp], f16, tag="x",
                                 name=f"xtile{pi}")
                    nc.sync.dma_start(xn[:], xview(xop, Tp))
                    x_tiles[pi] = xn

                fill = (lambda m: l1_chain(si + 1, m)) if si + 1 < nseq \
                    else (lambda m: None)

                fill(0)

                # L2: z[LAT, T] = relu(W2.T @ h)
                htile = h_tiles.pop(si)
                ztile = zp.tile([128, T], f16, tag="z", name=f"z{si}")
                ps2 = ppb.tile([128, T], f32, tag="ps", name=f"ps2_{si}")
                for k in range(KC2):
                    nc.tensor.matmul(ps2[:],
                                     wt[e][:, W2O + k * LAT:W2O + (k + 1) * LAT],
                                     htile[:, k * T:(k + 1) * T],
                                     start=(k == 0), stop=(k == KC2 - 1))
                evict(ztile[:], ps2[:], True, e * NBC + MC1)

                fill(1)

                # L3: d[H1, T] = relu(W3.T @ z)  (K=128: single-matmul chunks)
                dtile = dp.tile([128, MC1 * T], f16, tag="d", name=f"d{si}")
                for m in range(MC1):
                    pool = ppb if m == MC1 - 1 else ppc
                    ps3 = pool.tile([128, T], f32, tag="ps",
                                    name=f"ps3_{si}_{m}")
                    nc.tensor.matmul(ps3[:],
                                     wt[e][:, W3O + m * 128:W3O + (m + 1) * 128],
                                     ztile[:], start=True, stop=True)
                    evict(dtile[:, m * T:(m + 1) * T], ps3[:], True,
                          e * NBC + MC1 + 1 + m)

                fill(2)

                # L4: y[D_OUT, T] = W4.T @ d (no relu), chains of 4
                ytile = yp.tile([128, MC4 * T], f16, tag="y", name=f"y{si}")
                yv = yt[yo:yo + 128 * MC4 * T].rearrange("(p f) -> p f",
                                                         p=128)
                for m in range(MC4):
                    ps4 = ppc.tile([128, T], f32, tag="ps",
                                   name=f"ps4_{si}_{m}")
                    for k in range(KC4):
                        nc.tensor.matmul(
                            ps4[:],
                            wt[e][:, W4O + k * D_OUT + m * 128:
                                  W4O + k * D_OUT + (m + 1) * 128],
                            dtile[:, k * T:(k + 1) * T],
                            start=(k == 0), stop=(k == KC4 - 1))
                    evict(ytile[:, m * T:(m + 1) * T], ps4[:], False,
                          e * NBC + 2 * MC1 + 1 + m)
                if si == nseq - 1:
                    # last tile: store in halves on the (idle by then) sync
                    # queue so the first half overlaps the m4-7 evictions
                    hcols = MC4 // 2 * T
                    nc.sync.dma_start(yv[:, :hcols], ytile[:, :hcols])
                    nc.sync.dma_start(yv[:, hcols:], ytile[:, hcols:])
                else:
                    # scalar HWDGE ring: software-queue (SWDGE) drains cost
                    # ~5us at teardown, hardware queues ~0.5us
                    nc.scalar.dma_start(yv[:], ytile[:])

                fill(3)

    nc.compile()
    return nc


def kernel(**inputs) -> np.ndarray:
    from concourse.bass_utils import run_bass_kernel_spmd

    X = np.ascontiguousarray(inputs["X"], dtype=np.float32)
    sym_ids = np.asarray(inputs["sym_ids"]).astype(np.int64).ravel()
    We = [inputs["We1"], inputs["We2"], inputs["Wd1"], inputs["Wd2"]]
    be = [np.asarray(inputs["be1"], dtype=np.float32),
          np.asarray(inputs["be2"], dtype=np.float32),
          np.asarray(inputs["bd1"], dtype=np.float32),
          np.asarray(inputs["bd2"], dtype=np.float32)]
    use_bias = any(np.any(b) for b in be)

    N, D_IN = X.shape
    E, _, H1 = We[0].shape
    LAT = We[1].shape[2]
    D_OUT = We[3].shape[2]
    KC1 = D_IN // 128
    MC1 = H1 // 128
    MC4 = D_OUT // 128
    NBC = MC1 + 1 + MC1 + MC4
    TMAX = 512

    # ---- host routing: per-expert, per-core index assignment ----
    core_idx = [[None] * E for _ in range(N_CORES)]
    C_e = [0] * E
    for e in range(E):
        idx = np.flatnonzero(sym_ids == e)
        n = len(idx)
        base, rem = divmod(n, N_CORES)
        s = 0
        for c in range(N_CORES):
            cnt = base + (1 if c < rem else 0)
            core_idx[c][e] = idx[s:s + cnt]
            s += cnt
        C_e[e] = base + (1 if rem else 0)

    # per-expert tiling: nt equal-width tiles (multiples of 8); the last tile
    # overlaps backwards into the previous one to absorb the rounding. Equal
    # widths avoid narrow tiles, whose matmuls expose the ~128-cycle PE
    # pipeline drain per instruction.
    tiles = []
    for e in range(E):
        ce = max(C_e[e], 1)
        nt = -(-ce // TMAX)
        Tw = -(-ce // nt)
        T = -(-Tw // 8) * 8
        tiles.append(tuple([T] * nt))

    # ---- build / fetch compiled program ----
    dims = (D_IN, H1, LAT, D_OUT, E)
    n_bias_cols = E * NBC
    key = (dims, tuple(tiles), use_bias)
    nc = _PROGRAM_CACHE.get(key)
    if nc is None:
        nc = _build_program(dims, tiles, use_bias, n_bias_cols)
        _PROGRAM_CACHE[key] = nc

    # ---- prepare inputs ----
    XrT = np.ascontiguousarray(X.astype(np.float16).T)       # [D_IN, N]

    # weights in device layout: [E, 128, kc*mw] (k-chunk-major columns)
    def wdev(w, kc, mw):
        return np.ascontiguousarray(
            np.asarray(w, dtype=np.float32).astype(np.float16)
            .reshape(E, kc, 128, mw).transpose(0, 2, 1, 3)
            .reshape(E, 128, kc * mw))

    Wall = np.ascontiguousarray(np.concatenate(
        [wdev(We[0], KC1, H1), wdev(We[1], H1 // 128, LAT),
         wdev(We[2], LAT // 128, H1), wdev(We[3], H1 // 128, D_OUT)],
        axis=2))

    bias_h = None
    if use_bias:
        bias_h = np.zeros((128, n_bias_cols), np.float32)
        for e in range(E):
            col = e * NBC
            for b in (be[0][e], be[1][e], be[2][e], be[3][e]):
                for mch in range(len(b) // 128):
                    bias_h[:, col] = b[mch * 128:(mch + 1) * 128]
                    col += 1

    seq, x_total, y_total = _plan(dims, tiles)

    perms = []
    in_maps = []
    for c in range(N_CORES):
        xflat = np.empty(x_total, dtype=np.float16)
        perm = []
        for e, t, T, xo, yo in seq:
            idx = core_idx[c][e]
            ce = len(idx)
            start = max(min(t * T, ce - T), 0)
            rows = idx[start:start + T]
            nvalid = len(rows)
            if nvalid < T:  # tiny core count: pad with first atom
                rows = np.concatenate(
                    [rows, np.full(T - nvalid, idx[0] if ce else 0,
                                   np.int64)])
            perm.append((rows, nvalid))
            # [D_IN, T] -> [128, KC1, T] (k-chunk major per partition)
            g = XrT[:, rows].reshape(KC1, 128, T)
            xflat[xo:xo + 128 * KC1 * T] = (
                g.transpose(1, 0, 2).reshape(-1))
        perms.append(perm)
        m = {"xt": xflat, "wall": Wall}
        if use_bias:
            m["bias"] = bias_h
        in_maps.append(m)

    res = run_bass_kernel_spmd(nc, in_maps, core_ids=list(range(N_CORES)),
                               trace=_TRACE)
    _LAST["res"] = res

    # ---- unshard ----
    Y = np.empty((N, D_OUT), dtype=np.float32)
    for c in range(N_CORES):
        yflat = res.results[c]["yt"]
        for ti, (e, t, T, xo, yo) in enumerate(seq):
            ytc = (yflat[yo:yo + 128 * MC4 * T].astype(np.float32)
                   .reshape(128, MC4, T).transpose(1, 0, 2)
                   .reshape(D_OUT, T))
            rows, nvalid = perms[c][ti]
            Y[rows[:nvalid]] = ytc.T[:nvalid]
    return Y
